# revision 1
# baseline (speedup 1.0000x reference)
"""Multi-head attention (B=4, S=2048, D=1024, H=16, causal) on 8 NeuronCores.

Sharding: data-parallel over batch (4) x tensor-parallel over head halves (2).
Core c handles batch c//2 with heads (c%2)*8 .. (c%2)*8+7 and produces output
columns (c%2)*512 .. +512 after an in-pair AllGather of the attention output.

Device pipeline (all bf16 matmuls, fp32 PSUM accumulation):
  phase 1: QKV projections from host-pre-transposed inputs -> QT/KT [i, s],
           V1 [s, (ones|V)] per head with a 64-wide ones block so the PV
           matmul produces softmax denominators replicated on 64 partitions.
  phase 2: per head pair / 512-query block: logits^T tiles [sk=128, sq=512]
           via K=64 row-packed matmuls (2 heads concurrent on the PE), exp on
           ScalarE (scale=1/8 fused), causal handled by restricting computed
           ranges + a triangular-mask multiply on the diagonal 128x128 tile,
           PV matmul with full M=128 (64 value cols + 64 ones cols).
  phase 3: pair-wise AllGather of attnT (issued per-pair, overlapped), then
           the output projection for this core's 512 output columns.
"""

import numpy as np
import ml_dtypes

import concourse.bass as bass
import concourse.mybir as mybir
import concourse.tile as tile
from concourse import bacc
from concourse.bass_utils import run_bass_kernel_spmd

B, S, D, H = 4, 2048, 1024, 16
HD = D // H  # 64
NCORES = 8
HH = D // 2  # 512 = head-half width (8 heads x 64) = output col split
BF16 = mybir.dt.bfloat16
F32 = mybir.dt.float32
NPBF = ml_dtypes.bfloat16

P = 128          # partitions
NB = S // 512    # 4 query/seq blocks of 512
NT = S // P      # 16 seq tiles of 128
NC = D // P      # 8 contraction chunks of 128
NPAIR = 4        # head pairs per core

REPLICA_GROUPS = [[0, 1], [2, 3], [4, 5], [6, 7]]

_cache = {}


def _build():
    nc = bacc.Bacc("TRN2", target_bir_lowering=False, debug=False,
                   num_devices=NCORES)

    # ---- dram I/O ----
    qT = nc.dram_tensor("qT", [P, NC, S], BF16, kind="ExternalInput")
    kT = nc.dram_tensor("kT", [P, NC, S], BF16, kind="ExternalInput")
    vT = nc.dram_tensor("vT", [P, NC, S], BF16, kind="ExternalInput")
    wqT = nc.dram_tensor("wqT", [P, NC, HH], BF16, kind="ExternalInput")
    wkT = nc.dram_tensor("wkT", [P, NC, HH], BF16, kind="ExternalInput")
    wvT = nc.dram_tensor("wvT", [P, NC, HH], BF16, kind="ExternalInput")
    woT = nc.dram_tensor("woT", [P, NC, HH], BF16, kind="ExternalInput")
    bqv = nc.dram_tensor("bq", [1, HH], BF16, kind="ExternalInput")
    bkv = nc.dram_tensor("bk", [1, HH], BF16, kind="ExternalInput")
    bvv = nc.dram_tensor("bv", [1, HH], BF16, kind="ExternalInput")
    bov = nc.dram_tensor("bo", [1, HH], BF16, kind="ExternalInput")
    onesd = nc.dram_tensor("ones", [1, 512], BF16, kind="ExternalInput")
    trid = nc.dram_tensor("tri", [P, P], BF16, kind="ExternalInput")
    out = nc.dram_tensor("out", [S, HH], F32, kind="ExternalOutput")

    with tile.TileContext(nc) as tc:
        with (
            tc.tile_pool(name="persist", bufs=1) as pp,
            tc.tile_pool(name="dram", bufs=1, space="DRAM") as dp,
        ):
            # persistent sbuf tensors
            wq_sb = pp.tile([P, NC, HH], BF16, tag="wq")
            wk_sb = pp.tile([P, NC, HH], BF16, tag="wk")
            wv_sb = pp.tile([P, NC, HH], BF16, tag="wv")
            wo_sb = pp.tile([P, NC, HH], BF16, tag="wo")
            bq_sb = pp.tile([1, HH], BF16, tag="bq")
            bk_sb = pp.tile([1, HH], BF16, tag="bk")
            bv_sb = pp.tile([1, HH], BF16, tag="bv")
            bo_sb = pp.tile([1, HH], BF16, tag="bo")
            ones_sb = pp.tile([1, 512], BF16, tag="ones")
            tri_sb = pp.tile([P, P], BF16, tag="tri")
            QT = [pp.tile([P, S], BF16, tag=f"qt{p}", name=f"qt{p}")
                  for p in range(NPAIR)]
            KT = [pp.tile([P, S], BF16, tag=f"kt{p}", name=f"kt{p}")
                  for p in range(NPAIR)]
            # V1[s-part, s-tile, head, 128]: even head -> V in cols 0:64,
            # ones in 64:128; odd head -> ones in 0:64, V in 64:128.
            V1 = pp.tile([P, NT, 8, P], BF16, tag="v1")
            atf = pp.tile([P, NC, S], BF16, tag="atf")
            attnT = [pp.tile([P, S], BF16, tag=f"at{p}", name=f"at{p}")
                     for p in range(NPAIR)]

            own_dram = [dp.tile([P, S], BF16, tag=f"own{p}", name=f"own{p}")
                        for p in range(NPAIR)]
            all_dram = [dp.tile([2, P, S], BF16, tag=f"all{p}", name=f"all{p}")
                        for p in range(NPAIR)]

            for w_sb, w_d in ((wq_sb, wqT), (wk_sb, wkT), (wv_sb, wvT),
                              (wo_sb, woT)):
                nc.sync.dma_start(out=w_sb[:], in_=w_d.ap())
            for b_sb, b_d in ((bq_sb, bqv), (bk_sb, bkv), (bv_sb, bvv),
                              (bo_sb, bov), (ones_sb, onesd), (tri_sb, trid)):
                nc.sync.dma_start(out=b_sb[:], in_=b_d.ap())

            # ones blocks of V1 (written once; V drains only touch value cols)
            nc.vector.memset(V1[:, :, 0:8:2, 64:128], 1.0)
            nc.vector.memset(V1[:, :, 1:8:2, 0:64], 1.0)

            # ---------------- phase 1: k/v projections ----------------
            with tc.tile_pool(name="xt", bufs=1) as xtp:
              with tc.tile_pool(name="proj_ps", bufs=8, space="PSUM") as pps:
                for x_d, w_sb, b_sb, kind in (
                    (kT, wk_sb, bk_sb, "k"),
                    (vT, wv_sb, bv_sb, "v"),
                    (qT, wq_sb, bq_sb, "q"),
                ):
                    x_sb = [xtp.tile([P, S], BF16, tag=f"xt{c}",
                                     name=f"xt{c}") for c in range(NC)]
                    for c in range(NC):
                        nc.sync.dma_start(out=x_sb[c][:], in_=x_d.ap()[:, c, :])
                    for it in range(4):
                        ps = [pps.tile([P, 512], F32, tag="proj", name=f"proj{sb}")
                              for sb in range(4)]
                        for c in range(NC):
                            for sb in range(4):
                                if kind == "v":
                                    st = it * 4 + sb
                                    nc.tensor.matmul(
                                        ps[sb][:],
                                        lhsT=x_sb[c][:, st * P:(st + 1) * P],
                                        rhs=w_sb[:, c, :],
                                        start=(c == 0), stop=False)
                                else:
                                    nc.tensor.matmul(
                                        ps[sb][:],
                                        lhsT=w_sb[:, c, it * P:(it + 1) * P],
                                        rhs=x_sb[c][:, sb * 512:(sb + 1) * 512],
                                        start=(c == 0), stop=False)
                        for sb in range(4):
                            if kind == "v":
                                nc.tensor.matmul(
                                    ps[sb][:], lhsT=ones_sb[0:1, 0:P],
                                    rhs=b_sb[0:1, :],
                                    start=False, stop=True)
                            else:
                                nc.tensor.matmul(
                                    ps[sb][:],
                                    lhsT=b_sb[0:1, it * P:(it + 1) * P],
                                    rhs=ones_sb[0:1, 0:512],
                                    start=False, stop=True)
                        # drains
                        for sb in range(4):
                            if kind == "v":
                                st = it * 4 + sb
                                pv3 = ps[sb][:].rearrange(
                                    "p (h d) -> p h d", h=8)
                                nc.scalar.copy(
                                    out=V1[:, st, 0:8:2, 0:64],
                                    in_=pv3[:, 0:8:2, :])
                                nc.scalar.copy(
                                    out=V1[:, st, 1:8:2, 64:128],
                                    in_=pv3[:, 1:8:2, :])
                            else:
                                dst = QT if kind == "q" else KT
                                nc.scalar.copy(
                                    out=dst[it][:, sb * 512:(sb + 1) * 512],
                                    in_=ps[sb][:])

              # ------ phase 2: attention + interleaved q projections ------
              with (
                tc.tile_pool(name="pt", bufs=3) as ptp,
                tc.tile_pool(name="rec", bufs=2) as rcp,
                tc.tile_pool(name="lg_ps", bufs=2, space="PSUM") as lgp,
                tc.tile_pool(name="pv_ps", bufs=4, space="PSUM") as pvp,
              ):
                for pr in range(NPAIR):
                    for qb in range(NB):
                        pvA = pvp.tile([P, 512], F32, tag="pv")
                        pvB = pvp.tile([P, 512], F32, tag="pv")
                        nch = qb * 4 + 4
                        q0 = qb * 512

                        def emit_pv(c, pt, off, pvA=pvA, pvB=pvB, pr=pr,
                                    nch=nch):
                            nc.tensor.matmul(
                                pvA[:, off:512],
                                lhsT=V1[:, c, 2 * pr, :],
                                rhs=pt[:, off:512],
                                start=(c == 0), stop=(c == nch - 1))
                            nc.tensor.matmul(
                                pvB[:, off:512],
                                lhsT=V1[:, c, 2 * pr + 1, :],
                                rhs=pt[:, 512 + off:1024],
                                start=(c == 0), stop=(c == nch - 1))

                        pending = None  # (c, pt, off) awaiting PV emission
                        for c in range(nch):
                            jj = c - qb * 4
                            off = 128 * jj if jj > 0 else 0
                            lg = lgp.tile([P, 1024], F32, tag="lg")
                            nc.tensor.matmul(
                                lg[:, 0:512],
                                lhsT=KT[pr][0:64, c * P:(c + 1) * P],
                                rhs=QT[pr][0:64, q0:q0 + 512],
                                start=True, stop=True, tile_position=(0, 0))
                            nc.tensor.matmul(
                                lg[:, 512:1024],
                                lhsT=KT[pr][64:128, c * P:(c + 1) * P],
                                rhs=QT[pr][64:128, q0:q0 + 512],
                                start=True, stop=True, tile_position=(64, 0))
                            pt = ptp.tile([P, 1024], BF16, tag="pt")
                            lg3 = lg[:].rearrange("p (h n) -> p h n", h=2)
                            pt3 = pt[:].rearrange("p (h n) -> p h n", h=2)
                            nc.scalar.activation(
                                out=pt3[:, :, off:512],
                                in_=lg3[:, :, off:512],
                                func=mybir.ActivationFunctionType.Exp,
                                scale=0.125)
                            if jj >= 0:  # diagonal 128x128: causal mask
                                nc.vector.tensor_mul(
                                    out=pt[:, off:off + P],
                                    in0=pt[:, off:off + P], in1=tri_sb[:])
                                nc.vector.tensor_mul(
                                    out=pt[:, 512 + off:512 + off + P],
                                    in0=pt[:, 512 + off:512 + off + P],
                                    in1=tri_sb[:])
                            if pending is not None:
                                emit_pv(*pending)
                            pending = (c, pt, off)
                        emit_pv(*pending)
                        # drains: even head value rows 0:64, denom 64:128;
                        # odd head value rows 64:128, denom 0:64.
                        rec = rcp.tile([P, 1024], F32, tag="rec")
                        nc.vector.reciprocal(rec[64:128, 0:512],
                                             pvA[64:128, :])
                        nc.sync.dma_start(out=rec[0:64, 0:512],
                                          in_=rec[64:128, 0:512])
                        nc.vector.tensor_mul(
                            out=attnT[pr][0:64, q0:q0 + 512],
                            in0=pvA[0:64, :], in1=rec[0:64, 0:512])
                        nc.vector.reciprocal(rec[0:64, 512:1024],
                                             pvB[0:64, :])
                        nc.sync.dma_start(out=rec[64:128, 512:1024],
                                          in_=rec[0:64, 512:1024])
                        nc.vector.tensor_mul(
                            out=attnT[pr][64:128, q0:q0 + 512],
                            in0=pvB[64:128, :], in1=rec[64:128, 512:1024])
                    # pair done: exchange with partner core
                    nc.sync.dma_start(out=own_dram[pr][:], in_=attnT[pr][:])
                    nc.gpsimd.collective_compute(
                        "AllGather", mybir.AluOpType.bypass,
                        replica_groups=REPLICA_GROUPS,
                        ins=[own_dram[pr].opt()],
                        outs=[all_dram[pr].opt()])
                    for hf in range(2):
                        nc.sync.dma_start(out=atf[:, hf * 4 + pr, :],
                                          in_=all_dram[pr][hf, :, :])

            # ---------------- phase 3: output projection ----------------
            with (
                tc.tile_pool(name="ob", bufs=3) as obp,
                tc.tile_pool(name="fc_ps", bufs=8, space="PSUM") as fcp,
            ):
                for st in range(NT):
                    fc = fcp.tile([P, 512], F32, tag="fc")
                    for ic in range(NC - 1):
                        nc.tensor.matmul(
                            fc[:],
                            lhsT=atf[:, ic, st * P:(st + 1) * P],
                            rhs=wo_sb[:, ic, :],
                            start=(ic == 0), stop=False)
                    nc.tensor.matmul(fc[:], lhsT=ones_sb[0:1, 0:P],
                                     rhs=bo_sb[0:1, :], start=False, stop=False)
                    nc.tensor.matmul(
                        fc[:],
                        lhsT=atf[:, NC - 1, st * P:(st + 1) * P],
                        rhs=wo_sb[:, NC - 1, :],
                        start=False, stop=True)
                    ob = obp.tile([P, 512], F32, tag="ob")
                    nc.vector.tensor_copy(out=ob[:], in_=fc[:])
                    nc.sync.dma_start(
                        out=out.ap()[st * P:(st + 1) * P, :], in_=ob[:])

    nc.compile()
    return nc


def _get_nc():
    if "nc" not in _cache:
        _cache["nc"] = _build()
    return _cache["nc"]


def _chunked(xT):
    # [D, cols] -> [128, NC, cols] so each partition's data is contiguous
    cols = xT.shape[1]
    return np.ascontiguousarray(
        xT.reshape(NC, P, cols).transpose(1, 0, 2)).astype(NPBF)


def _prep_inputs(q, k, v, Wq, bq, Wk, bk, Wv, bv, Wo, bo):
    ones = np.ones((1, 512), NPBF)
    tri = np.triu(np.ones((P, P), np.float32)).astype(NPBF)
    in_maps = []
    for c in range(NCORES):
        b, hf = divmod(c, 2)
        hs = slice(hf * HH, (hf + 1) * HH)
        in_maps.append({
            "qT": _chunked(q[b].T),
            "kT": _chunked(k[b].T),
            "vT": _chunked(v[b].T),
            "wqT": _chunked(Wq[hs].T),
            "wkT": _chunked(Wk[hs].T),
            "wvT": _chunked(Wv[hs].T),
            "woT": _chunked(Wo[hs].T),
            "bq": bq[hs].reshape(1, HH).astype(NPBF),
            "bk": bk[hs].reshape(1, HH).astype(NPBF),
            "bv": bv[hs].reshape(1, HH).astype(NPBF),
            "bo": bo[hs].reshape(1, HH).astype(NPBF),
            "ones": ones,
            "tri": tri,
        })
    return in_maps


def kernel(q, k, v, mask, Wq, bq, Wk, bk, Wv, bv, Wo, bo, _trace=False):
    q, k, v = (np.asarray(x, np.float32) for x in (q, k, v))
    mask = np.asarray(mask, np.float32)
    exp_mask = np.triu(np.ones((S, S), np.float32), k=1)[None, None]
    assert mask.shape == (1, 1, S, S) and np.array_equal(mask, exp_mask), \
        "kernel specialized for the causal mask produced by setup_inputs()"

    nc = _get_nc()
    in_maps = _prep_inputs(q, k, v, Wq, bq, Wk, bk, Wv, bv, Wo, bo)
    res = run_bass_kernel_spmd(nc, in_maps, list(range(NCORES)),
                               trace=_trace)
    if _trace:
        _cache["last_result"] = res
    full = np.empty((B, S, D), np.float32)
    for c in range(NCORES):
        b, hf = divmod(c, 2)
        full[b, :, hf * HH:(hf + 1) * HH] = res.results[c]["out"]
    return full



# revision 11
# speedup vs baseline: 1.1082x; 1.1082x over previous
"""Multi-head attention (B=4, S=2048, D=1024, H=16, causal) on 8 NeuronCores.

Sharding: data-parallel over batch (4) x tensor-parallel over head halves (2).
Core c handles batch c//2 with heads (c%2)*8 .. (c%2)*8+7 and produces output
columns (c%2)*512 .. +512 after an in-pair AllGather of the attention output.

Device pipeline (all bf16 matmuls, fp32 PSUM accumulation):
  phase 1: QKV projections from host-pre-transposed inputs -> QT/KT [i, s],
           V1 [s, (ones|V)] per head with a 64-wide ones block so the PV
           matmul produces softmax denominators replicated on 64 partitions.
  phase 2: per head pair / 512-query block: logits^T tiles [sk=128, sq=512]
           via K=64 row-packed matmuls (2 heads concurrent on the PE), exp on
           ScalarE (scale=1/8 fused), causal handled by restricting computed
           ranges + a triangular-mask multiply on the diagonal 128x128 tile,
           PV matmul with full M=128 (64 value cols + 64 ones cols).
  phase 3: pair-wise AllGather of attnT (issued per-pair, overlapped), then
           the output projection for this core's 512 output columns.
"""

import numpy as np
import ml_dtypes

import concourse.bass as bass
import concourse.mybir as mybir
import concourse.tile as tile
from concourse import bacc
from concourse.bass_utils import run_bass_kernel_spmd

B, S, D, H = 4, 2048, 1024, 16
HD = D // H  # 64
NCORES = 8
HH = D // 2  # 512 = head-half width (8 heads x 64) = output col split
BF16 = mybir.dt.bfloat16
F32 = mybir.dt.float32
NPBF = ml_dtypes.bfloat16

P = 128          # partitions
NB = S // 512    # 4 query/seq blocks of 512
NT = S // P      # 16 seq tiles of 128
NC = D // P      # 8 contraction chunks of 128
NPAIR = 4        # head pairs per core

REPLICA_GROUPS = [[0, 1], [2, 3], [4, 5], [6, 7]]

_cache = {}


def _build():
    nc = bacc.Bacc("TRN2", target_bir_lowering=False, debug=False,
                   num_devices=NCORES)

    # ---- dram I/O ----
    # bq folded into the Q-projection drain (per-partition activation bias);
    # bk/bv cancel mathematically (softmax shift-invariance / sum(w)=1) and
    # bo is folded host-side into bo_tile = bo[hs] + bv @ Wo[hs].T.
    qT = nc.dram_tensor("qT", [P, NC, S], BF16, kind="ExternalInput")
    kT = nc.dram_tensor("kT", [P, NC, S], BF16, kind="ExternalInput")
    vT = nc.dram_tensor("vT", [P, NC, S], BF16, kind="ExternalInput")
    wqT = nc.dram_tensor("wqT", [P, NC, HH], BF16, kind="ExternalInput")
    wkT = nc.dram_tensor("wkT", [P, NC, HH], BF16, kind="ExternalInput")
    wvT = nc.dram_tensor("wvT", [P, NC, HH], BF16, kind="ExternalInput")
    woT = nc.dram_tensor("woT", [P, NC, HH], BF16, kind="ExternalInput")
    bqv = nc.dram_tensor("bq", [P, 4], F32, kind="ExternalInput")
    bov = nc.dram_tensor("bo", [P, HH], F32, kind="ExternalInput")
    trid = nc.dram_tensor("tri", [P, P], BF16, kind="ExternalInput")
    out = nc.dram_tensor("out", [S, HH], F32, kind="ExternalOutput")

    with tile.TileContext(nc) as tc:
        with (
            tc.tile_pool(name="persist", bufs=1) as pp,
            tc.tile_pool(name="dram", bufs=1, space="DRAM") as dp,
        ):
            # persistent sbuf tensors
            wq_sb = pp.tile([P, NC, HH], BF16, tag="wq")
            wk_sb = pp.tile([P, NC, HH], BF16, tag="wk")
            wv_sb = pp.tile([P, NC, HH], BF16, tag="wv")
            wo_sb = pp.tile([P, NC, HH], BF16, tag="wo")
            bq_sb = pp.tile([P, 4], F32, tag="bq")
            bo_sb = pp.tile([P, HH], F32, tag="bo")
            tri_sb = pp.tile([P, P], BF16, tag="tri")
            QT = [pp.tile([P, S], BF16, tag=f"qt{p}", name=f"qt{p}")
                  for p in range(NPAIR)]
            KT = [pp.tile([P, S], BF16, tag=f"kt{p}", name=f"kt{p}")
                  for p in range(NPAIR)]
            # V1[s-part, s-tile, head, 128]: even head -> V in cols 0:64,
            # ones in 64:128; odd head -> ones in 0:64, V in 64:128.
            V1 = pp.tile([P, NT, 8, P], BF16, tag="v1")
            atf = pp.tile([P, NC, S], BF16, tag="atf")
            attnT = [pp.tile([P, S], BF16, tag=f"at{p}", name=f"at{p}")
                     for p in range(NPAIR)]

            own_dram = [dp.tile([P, S], BF16, tag=f"own{p}", name=f"own{p}")
                        for p in range(NPAIR)]
            all_dram = [dp.tile([2, P, S], BF16, tag=f"all{p}", name=f"all{p}")
                        for p in range(NPAIR)]

            # per-chunk weight DMAs so the first matmuls wait on 128KB, not
            # 1MB; k first (used first), wo last.
            for w_sb, w_d in ((wk_sb, wkT), (wv_sb, wvT), (wq_sb, wqT),
                              (wo_sb, woT)):
                for c in range(NC):
                    nc.sync.dma_start(out=w_sb[:, c, :], in_=w_d.ap()[:, c, :])
            for b_sb, b_d in ((bq_sb, bqv), (bo_sb, bov), (tri_sb, trid)):
                nc.sync.dma_start(out=b_sb[:], in_=b_d.ap())

            # ones blocks of V1 (written once; V drains only touch value cols)
            nc.vector.memset(V1[:, :, 0:8:2, 64:128], 1.0)
            nc.vector.memset(V1[:, :, 1:8:2, 0:64], 1.0)

            # ---------------- phase 1: k/v projections ----------------
            with tc.tile_pool(name="xt", bufs=1) as xtp:
              with tc.tile_pool(name="proj_ps", bufs=8, space="PSUM") as pps:
                for x_d, w_sb, kind in (
                    (kT, wk_sb, "k"),
                    (vT, wv_sb, "v"),
                    (qT, wq_sb, "q"),
                ):
                    x_sb = [xtp.tile([P, S], BF16, tag=f"xt{c}",
                                     name=f"xt{c}") for c in range(NC)]
                    for c in range(NC):
                        nc.sync.dma_start(out=x_sb[c][:], in_=x_d.ap()[:, c, :])
                    for it in range(4):
                        ps = [pps.tile([P, 512], F32, tag="proj", name=f"proj{sb}")
                              for sb in range(4)]
                        for c in range(NC):
                            for sb in range(4):
                                if kind == "v":
                                    st = it * 4 + sb
                                    nc.tensor.matmul(
                                        ps[sb][:],
                                        lhsT=x_sb[c][:, st * P:(st + 1) * P],
                                        rhs=w_sb[:, c, :],
                                        start=(c == 0), stop=(c == NC - 1))
                                else:
                                    nc.tensor.matmul(
                                        ps[sb][:],
                                        lhsT=w_sb[:, c, it * P:(it + 1) * P],
                                        rhs=x_sb[c][:, sb * 512:(sb + 1) * 512],
                                        start=(c == 0), stop=(c == NC - 1))
                        # drains
                        for sb in range(4):
                            if kind == "v":
                                st = it * 4 + sb
                                pv3 = ps[sb][:].rearrange(
                                    "p (h d) -> p h d", h=8)
                                nc.scalar.copy(
                                    out=V1[:, st, 0:8:2, 0:64],
                                    in_=pv3[:, 0:8:2, :])
                                nc.scalar.copy(
                                    out=V1[:, st, 1:8:2, 64:128],
                                    in_=pv3[:, 1:8:2, :])
                            elif kind == "k":
                                nc.scalar.copy(
                                    out=KT[it][:, sb * 512:(sb + 1) * 512],
                                    in_=ps[sb][:])
                            else:
                                nc.scalar.activation(
                                    out=QT[it][:, sb * 512:(sb + 1) * 512],
                                    in_=ps[sb][:],
                                    func=mybir.ActivationFunctionType.Identity,
                                    bias=bq_sb[:, it:it + 1])

              # ------ phase 2: attention + interleaved q projections ------
              with (
                tc.tile_pool(name="pt", bufs=3) as ptp,
                tc.tile_pool(name="rec", bufs=2) as rcp,
                tc.tile_pool(name="lg_ps", bufs=2, space="PSUM") as lgp,
                tc.tile_pool(name="pv_ps", bufs=4, space="PSUM") as pvp,
              ):
                for pr in range(NPAIR):
                    for qb in range(NB):
                        pvA = pvp.tile([P, 512], F32, tag="pv")
                        pvB = pvp.tile([P, 512], F32, tag="pv")
                        nch = qb * 4 + 4
                        q0 = qb * 512

                        def emit_pv(c, pt, off, pvA=pvA, pvB=pvB, pr=pr,
                                    nch=nch):
                            nc.tensor.matmul(
                                pvA[:, off:512],
                                lhsT=V1[:, c, 2 * pr, :],
                                rhs=pt[:, off:512],
                                start=(c == 0), stop=(c == nch - 1))
                            nc.tensor.matmul(
                                pvB[:, off:512],
                                lhsT=V1[:, c, 2 * pr + 1, :],
                                rhs=pt[:, 512 + off:1024],
                                start=(c == 0), stop=(c == nch - 1))

                        pending = None  # (c, pt, off) awaiting PV emission
                        for c in range(nch):
                            jj = c - qb * 4
                            off = 128 * jj if jj > 0 else 0
                            lg = lgp.tile([P, 1024], F32, tag="lg")
                            nc.tensor.matmul(
                                lg[:, off:512],
                                lhsT=KT[pr][0:64, c * P:(c + 1) * P],
                                rhs=QT[pr][0:64, q0 + off:q0 + 512],
                                start=True, stop=True, tile_position=(0, 0))
                            nc.tensor.matmul(
                                lg[:, 512 + off:1024],
                                lhsT=KT[pr][64:128, c * P:(c + 1) * P],
                                rhs=QT[pr][64:128, q0 + off:q0 + 512],
                                start=True, stop=True, tile_position=(64, 0))
                            pt = ptp.tile([P, 1024], BF16, tag="pt")
                            lg3 = lg[:].rearrange("p (h n) -> p h n", h=2)
                            pt3 = pt[:].rearrange("p (h n) -> p h n", h=2)
                            nc.scalar.activation(
                                out=pt3[:, :, off:512],
                                in_=lg3[:, :, off:512],
                                func=mybir.ActivationFunctionType.Exp,
                                scale=0.125)
                            if jj >= 0:  # diagonal 128x128: causal mask
                                nc.vector.tensor_mul(
                                    out=pt[:, off:off + P],
                                    in0=pt[:, off:off + P], in1=tri_sb[:])
                                nc.vector.tensor_mul(
                                    out=pt[:, 512 + off:512 + off + P],
                                    in0=pt[:, 512 + off:512 + off + P],
                                    in1=tri_sb[:])
                            if pending is not None:
                                emit_pv(*pending)
                            pending = (c, pt, off)
                        emit_pv(*pending)
                        # drains: even head value rows 0:64, denom 64:128;
                        # odd head value rows 64:128, denom 0:64.
                        rec = rcp.tile([P, 1024], F32, tag="rec")
                        nc.vector.reciprocal(rec[64:128, 0:512],
                                             pvA[64:128, :])
                        nc.sync.dma_start(out=rec[0:64, 0:512],
                                          in_=rec[64:128, 0:512])
                        nc.vector.tensor_mul(
                            out=attnT[pr][0:64, q0:q0 + 512],
                            in0=pvA[0:64, :], in1=rec[0:64, 0:512])
                        nc.vector.reciprocal(rec[0:64, 512:1024],
                                             pvB[0:64, :])
                        nc.sync.dma_start(out=rec[64:128, 512:1024],
                                          in_=rec[0:64, 512:1024])
                        nc.vector.tensor_mul(
                            out=attnT[pr][64:128, q0:q0 + 512],
                            in0=pvB[64:128, :], in1=rec[64:128, 512:1024])
                    # pair done: exchange with partner core
                    nc.sync.dma_start(out=own_dram[pr][:], in_=attnT[pr][:])
                    nc.gpsimd.collective_compute(
                        "AllGather", mybir.AluOpType.bypass,
                        replica_groups=REPLICA_GROUPS,
                        ins=[own_dram[pr].opt()],
                        outs=[all_dram[pr].opt()])
                    for hf in range(2):
                        nc.sync.dma_start(out=atf[:, hf * 4 + pr, :],
                                          in_=all_dram[pr][hf, :, :])

            # ---------------- phase 3: output projection ----------------
            with (
                tc.tile_pool(name="ob", bufs=3) as obp,
                tc.tile_pool(name="fc_ps", bufs=8, space="PSUM") as fcp,
            ):
                for st in range(NT):
                    fc = fcp.tile([P, 512], F32, tag="fc")
                    for ic in range(NC):
                        nc.tensor.matmul(
                            fc[:],
                            lhsT=atf[:, ic, st * P:(st + 1) * P],
                            rhs=wo_sb[:, ic, :],
                            start=(ic == 0), stop=(ic == NC - 1))
                    ob = obp.tile([P, 512], F32, tag="ob")
                    nc.vector.tensor_add(out=ob[:], in0=fc[:], in1=bo_sb[:])
                    nc.sync.dma_start(
                        out=out.ap()[st * P:(st + 1) * P, :], in_=ob[:])

    nc.compile()
    return nc


def _get_nc():
    if "nc" not in _cache:
        _cache["nc"] = _build()
    return _cache["nc"]


def _chunked(xT):
    # [D, cols] -> [128, NC, cols] so each partition's data is contiguous
    cols = xT.shape[1]
    return np.ascontiguousarray(
        xT.reshape(NC, P, cols).transpose(1, 0, 2)).astype(NPBF)


def _prep_inputs(q, k, v, Wq, bq, Wk, bk, Wv, bv, Wo, bo):
    tri = np.triu(np.ones((P, P), np.float32)).astype(NPBF)
    in_maps = []
    for c in range(NCORES):
        b, hf = divmod(c, 2)
        hs = slice(hf * HH, (hf + 1) * HH)
        # bk drops (softmax shift-invariance); bv folds into bo because
        # softmax weights sum to 1: attn = sum(w*(v+bv)) = sum(w*v) + bv.
        bo_eff = (bo[hs].astype(np.float64)
                  + bv.astype(np.float64) @ Wo[hs].T.astype(np.float64))
        in_maps.append({
            "qT": _chunked(q[b].T),
            "kT": _chunked(k[b].T),
            "vT": _chunked(v[b].T),
            "wqT": _chunked(Wq[hs].T),
            "wkT": _chunked(Wk[hs].T),
            "wvT": _chunked(Wv[hs].T),
            "woT": _chunked(Wo[hs].T),
            "bq": np.ascontiguousarray(
                bq[hs].reshape(4, P).T).astype(np.float32),
            "bo": np.tile(bo_eff.astype(np.float32), (P, 1)),
            "tri": tri,
        })
    return in_maps


def kernel(q, k, v, mask, Wq, bq, Wk, bk, Wv, bv, Wo, bo, _trace=False):
    q, k, v = (np.asarray(x, np.float32) for x in (q, k, v))
    mask = np.asarray(mask, np.float32)
    exp_mask = np.triu(np.ones((S, S), np.float32), k=1)[None, None]
    assert mask.shape == (1, 1, S, S) and np.array_equal(mask, exp_mask), \
        "kernel specialized for the causal mask produced by setup_inputs()"

    nc = _get_nc()
    in_maps = _prep_inputs(q, k, v, Wq, bq, Wk, bk, Wv, bv, Wo, bo)
    res = run_bass_kernel_spmd(nc, in_maps, list(range(NCORES)),
                               trace=_trace)
    if _trace:
        _cache["last_result"] = res
    full = np.empty((B, S, D), np.float32)
    for c in range(NCORES):
        b, hf = divmod(c, 2)
        full[b, :, hf * HH:(hf + 1) * HH] = res.results[c]["out"]
    return full



# revision 13
# speedup vs baseline: 1.2225x; 1.1031x over previous
"""Multi-head attention (B=4, S=2048, D=1024, H=16, causal) on 8 NeuronCores.

Sharding: data-parallel over batch (4) x tensor-parallel over head halves (2).
Core c handles batch c//2 with heads (c%2)*8 .. (c%2)*8+7 and produces output
columns (c%2)*512 .. +512 after an in-pair AllGather of the attention output.

Device pipeline (all bf16 matmuls, fp32 PSUM accumulation):
  phase 1: QKV projections from host-pre-transposed inputs -> QT/KT [i, s],
           V1 [s, (ones|V)] per head with a 64-wide ones block so the PV
           matmul produces softmax denominators replicated on 64 partitions.
  phase 2: per head pair / 512-query block: logits^T tiles [sk=128, sq=512]
           via K=64 row-packed matmuls (2 heads concurrent on the PE), exp on
           ScalarE (scale=1/8 fused), causal handled by restricting computed
           ranges + a triangular-mask multiply on the diagonal 128x128 tile,
           PV matmul with full M=128 (64 value cols + 64 ones cols).
  phase 3: pair-wise AllGather of attnT (issued per-pair, overlapped), then
           the output projection for this core's 512 output columns.
"""

import numpy as np
import ml_dtypes

import concourse.bass as bass
import concourse.mybir as mybir
import concourse.tile as tile
from concourse import bacc
from concourse.bass_utils import run_bass_kernel_spmd

B, S, D, H = 4, 2048, 1024, 16
HD = D // H  # 64
NCORES = 8
HH = D // 2  # 512 = head-half width (8 heads x 64) = output col split
BF16 = mybir.dt.bfloat16
F32 = mybir.dt.float32
NPBF = ml_dtypes.bfloat16

P = 128          # partitions
NB = S // 512    # 4 query/seq blocks of 512
NT = S // P      # 16 seq tiles of 128
NC = D // P      # 8 contraction chunks of 128
NPAIR = 4        # head pairs per core

REPLICA_GROUPS = [[0, 1], [2, 3], [4, 5], [6, 7]]

_cache = {}


def _build():
    nc = bacc.Bacc("TRN2", target_bir_lowering=False, debug=False,
                   num_devices=NCORES)

    # ---- dram I/O ----
    # bq folded into the Q-projection drain (per-partition activation bias);
    # bk/bv cancel mathematically (softmax shift-invariance / sum(w)=1) and
    # bo is folded host-side into bo_tile = bo[hs] + bv @ Wo[hs].T.
    qT = nc.dram_tensor("qT", [P, NC, S], BF16, kind="ExternalInput")
    kT = nc.dram_tensor("kT", [P, NC, S], BF16, kind="ExternalInput")
    vT = nc.dram_tensor("vT", [P, NC, S], BF16, kind="ExternalInput")
    wqT = nc.dram_tensor("wqT", [P, NC, HH], BF16, kind="ExternalInput")
    wkT = nc.dram_tensor("wkT", [P, NC, HH], BF16, kind="ExternalInput")
    wvT = nc.dram_tensor("wvT", [P, NC, HH], BF16, kind="ExternalInput")
    woT = nc.dram_tensor("woT", [P, NC, HH], BF16, kind="ExternalInput")
    bqv = nc.dram_tensor("bq", [P, 4], F32, kind="ExternalInput")
    bov = nc.dram_tensor("bo", [P, HH], F32, kind="ExternalInput")
    trid = nc.dram_tensor("tri", [P, P], BF16, kind="ExternalInput")
    out = nc.dram_tensor("out", [S, HH], F32, kind="ExternalOutput")

    with tile.TileContext(nc) as tc:
        with (
            tc.tile_pool(name="persist", bufs=1) as pp,
            tc.tile_pool(name="dram", bufs=1, space="DRAM") as dp,
        ):
            # persistent sbuf tensors
            wq_sb = pp.tile([P, NC, HH], BF16, tag="wq")
            wk_sb = pp.tile([P, NC, HH], BF16, tag="wk")
            wv_sb = pp.tile([P, NC, HH], BF16, tag="wv")
            wo_sb = pp.tile([P, NC, HH], BF16, tag="wo")
            bq_sb = pp.tile([P, 4], F32, tag="bq")
            bo_sb = pp.tile([P, HH], F32, tag="bo")
            tri_sb = pp.tile([P, P], BF16, tag="tri")
            QT = [pp.tile([P, S], BF16, tag=f"qt{p}", name=f"qt{p}")
                  for p in range(NPAIR)]
            KT = [pp.tile([P, S], BF16, tag=f"kt{p}", name=f"kt{p}")
                  for p in range(NPAIR)]
            # V1[s-part, s-tile, head, 128]: even head -> V in cols 0:64,
            # ones in 64:128; odd head -> ones in 0:64, V in 64:128.
            V1 = pp.tile([P, NT, 8, P], BF16, tag="v1")
            atf = pp.tile([P, NC, S], BF16, tag="atf")
            attnT = [pp.tile([P, S], BF16, tag=f"at{p}", name=f"at{p}")
                     for p in range(NPAIR)]

            own_dram = [dp.tile([P, S], BF16, tag=f"own{p}", name=f"own{p}")
                        for p in range(NPAIR)]
            all_dram = [dp.tile([2, P, S], BF16, tag=f"all{p}", name=f"all{p}")
                        for p in range(NPAIR)]

            # per-chunk weight DMAs so the first matmuls wait on 128KB, not
            # 1MB; k first (used first), wo last.
            for w_sb, w_d in ((wk_sb, wkT), (wv_sb, wvT), (wq_sb, wqT),
                              (wo_sb, woT)):
                for c in range(NC):
                    nc.sync.dma_start(out=w_sb[:, c, :], in_=w_d.ap()[:, c, :])
            for b_sb, b_d in ((bq_sb, bqv), (bo_sb, bov), (tri_sb, trid)):
                nc.sync.dma_start(out=b_sb[:], in_=b_d.ap())

            # ones blocks of V1 (written once; V drains only touch value cols)
            nc.vector.memset(V1[:, :, 0:8:2, 64:128], 1.0)
            nc.vector.memset(V1[:, :, 1:8:2, 0:64], 1.0)

            # ---------------- phase 1: k/v projections ----------------
            with tc.tile_pool(name="xt", bufs=1) as xtp:
              with tc.tile_pool(name="proj_ps", bufs=8, space="PSUM") as pps:
                for x_d, w_sb, kind in (
                    (kT, wk_sb, "k"),
                    (vT, wv_sb, "v"),
                    (qT, wq_sb, "q"),
                ):
                    x_sb = [xtp.tile([P, S], BF16, tag=f"xt{c}",
                                     name=f"xt{c}") for c in range(NC)]
                    for c in range(NC):
                        nc.sync.dma_start(out=x_sb[c][:], in_=x_d.ap()[:, c, :])
                    for it in range(4):
                        ps = [pps.tile([P, 512], F32, tag="proj", name=f"proj{sb}")
                              for sb in range(4)]
                        for c in range(NC):
                            for sb in range(4):
                                if kind == "v":
                                    st = it * 4 + sb
                                    nc.tensor.matmul(
                                        ps[sb][:],
                                        lhsT=x_sb[c][:, st * P:(st + 1) * P],
                                        rhs=w_sb[:, c, :],
                                        start=(c == 0), stop=(c == NC - 1))
                                else:
                                    nc.tensor.matmul(
                                        ps[sb][:],
                                        lhsT=w_sb[:, c, it * P:(it + 1) * P],
                                        rhs=x_sb[c][:, sb * 512:(sb + 1) * 512],
                                        start=(c == 0), stop=(c == NC - 1))
                        # drains
                        for sb in range(4):
                            if kind == "v":
                                st = it * 4 + sb
                                pv3 = ps[sb][:].rearrange(
                                    "p (h d) -> p h d", h=8)
                                nc.scalar.copy(
                                    out=V1[:, st, 0:8:2, 0:64],
                                    in_=pv3[:, 0:8:2, :])
                                nc.scalar.copy(
                                    out=V1[:, st, 1:8:2, 64:128],
                                    in_=pv3[:, 1:8:2, :])
                            elif kind == "k":
                                nc.scalar.copy(
                                    out=KT[it][:, sb * 512:(sb + 1) * 512],
                                    in_=ps[sb][:])
                            else:
                                nc.scalar.activation(
                                    out=QT[it][:, sb * 512:(sb + 1) * 512],
                                    in_=ps[sb][:],
                                    func=mybir.ActivationFunctionType.Identity,
                                    bias=bq_sb[:, it:it + 1])

              # ------ phase 2: attention + interleaved q projections ------
              with (
                tc.tile_pool(name="pt", bufs=3) as ptp,
                tc.tile_pool(name="rec", bufs=2) as rcp,
                tc.tile_pool(name="lg_ps", bufs=2, space="PSUM") as lgp,
                tc.tile_pool(name="pv_ps", bufs=4, space="PSUM") as pvp,
              ):
                for pr in range(NPAIR):
                    for qb in range(NB):
                        pvA = pvp.tile([P, 512], F32, tag="pv")
                        pvB = pvp.tile([P, 512], F32, tag="pv")
                        nch = qb * 4 + 4
                        q0 = qb * 512

                        def emit_pv(c, pt, off, pvA=pvA, pvB=pvB, pr=pr,
                                    nch=nch):
                            nc.tensor.matmul(
                                pvA[:, off:512],
                                lhsT=V1[:, c, 2 * pr, :],
                                rhs=pt[:, off:512],
                                start=(c == 0), stop=(c == nch - 1))
                            nc.tensor.matmul(
                                pvB[:, off:512],
                                lhsT=V1[:, c, 2 * pr + 1, :],
                                rhs=pt[:, 512 + off:1024],
                                start=(c == 0), stop=(c == nch - 1))

                        pending = None  # (c, pt, off) awaiting PV emission
                        for c in range(nch):
                            jj = c - qb * 4
                            off = 128 * jj if jj > 0 else 0
                            lg = lgp.tile([P, 1024], F32, tag="lg")
                            nc.tensor.matmul(
                                lg[:, off:512],
                                lhsT=KT[pr][0:64, c * P:(c + 1) * P],
                                rhs=QT[pr][0:64, q0 + off:q0 + 512],
                                start=True, stop=True, tile_position=(0, 0))
                            nc.tensor.matmul(
                                lg[:, 512 + off:1024],
                                lhsT=KT[pr][64:128, c * P:(c + 1) * P],
                                rhs=QT[pr][64:128, q0 + off:q0 + 512],
                                start=True, stop=True, tile_position=(64, 0))
                            pt = ptp.tile([P, 1024], BF16, tag="pt")
                            lg3 = lg[:].rearrange("p (h n) -> p h n", h=2)
                            pt3 = pt[:].rearrange("p (h n) -> p h n", h=2)
                            nc.scalar.activation(
                                out=pt3[:, :, off:512],
                                in_=lg3[:, :, off:512],
                                func=mybir.ActivationFunctionType.Exp,
                                scale=0.125)
                            if jj >= 0:  # diagonal 128x128: causal mask
                                nc.vector.tensor_mul(
                                    out=pt[:, off:off + P],
                                    in0=pt[:, off:off + P], in1=tri_sb[:])
                                nc.vector.tensor_mul(
                                    out=pt[:, 512 + off:512 + off + P],
                                    in0=pt[:, 512 + off:512 + off + P],
                                    in1=tri_sb[:])
                            if pending is not None:
                                emit_pv(*pending)
                            pending = (c, pt, off)
                        emit_pv(*pending)
                        # drains: even head value rows 0:64, denom 64:128;
                        # odd head value rows 64:128, denom 0:64.
                        rec = rcp.tile([P, 1024], F32, tag="rec")
                        # approx_fast mishandles base partition 64: run the
                        # head-A recip over all 128 partitions (rows 0:64
                        # produce garbage recip-of-values, overwritten by the
                        # broadcast DMA below).
                        nc.vector.reciprocal_approx_fast(rec[:, 0:512],
                                                         pvA[:, :])
                        nc.sync.dma_start(out=rec[0:64, 0:512],
                                          in_=rec[64:128, 0:512])
                        nc.vector.tensor_mul(
                            out=attnT[pr][0:64, q0:q0 + 512],
                            in0=pvA[0:64, :], in1=rec[0:64, 0:512])
                        nc.vector.reciprocal_approx_fast(rec[0:64, 512:1024],
                                                         pvB[0:64, :])
                        nc.sync.dma_start(out=rec[64:128, 512:1024],
                                          in_=rec[0:64, 512:1024])
                        nc.vector.tensor_mul(
                            out=attnT[pr][64:128, q0:q0 + 512],
                            in0=pvB[64:128, :], in1=rec[64:128, 512:1024])
                    # pair done: exchange with partner core
                    nc.sync.dma_start(out=own_dram[pr][:], in_=attnT[pr][:])
                    nc.gpsimd.collective_compute(
                        "AllGather", mybir.AluOpType.bypass,
                        replica_groups=REPLICA_GROUPS,
                        ins=[own_dram[pr].opt()],
                        outs=[all_dram[pr].opt()])
                    for hf in range(2):
                        nc.sync.dma_start(out=atf[:, hf * 4 + pr, :],
                                          in_=all_dram[pr][hf, :, :])

            # ---------------- phase 3: output projection ----------------
            with (
                tc.tile_pool(name="ob", bufs=3) as obp,
                tc.tile_pool(name="fc_ps", bufs=8, space="PSUM") as fcp,
            ):
                for st in range(NT):
                    fc = fcp.tile([P, 512], F32, tag="fc")
                    for ic in range(NC):
                        nc.tensor.matmul(
                            fc[:],
                            lhsT=atf[:, ic, st * P:(st + 1) * P],
                            rhs=wo_sb[:, ic, :],
                            start=(ic == 0), stop=(ic == NC - 1))
                    ob = obp.tile([P, 512], F32, tag="ob")
                    nc.vector.tensor_add(out=ob[:], in0=fc[:], in1=bo_sb[:])
                    nc.sync.dma_start(
                        out=out.ap()[st * P:(st + 1) * P, :], in_=ob[:])

    nc.compile()
    return nc


def _get_nc():
    if "nc" not in _cache:
        _cache["nc"] = _build()
    return _cache["nc"]


def _chunked(xT):
    # [D, cols] -> [128, NC, cols] so each partition's data is contiguous
    cols = xT.shape[1]
    return np.ascontiguousarray(
        xT.reshape(NC, P, cols).transpose(1, 0, 2)).astype(NPBF)


def _prep_inputs(q, k, v, Wq, bq, Wk, bk, Wv, bv, Wo, bo):
    tri = np.triu(np.ones((P, P), np.float32)).astype(NPBF)
    in_maps = []
    for c in range(NCORES):
        b, hf = divmod(c, 2)
        hs = slice(hf * HH, (hf + 1) * HH)
        # bk drops (softmax shift-invariance); bv folds into bo because
        # softmax weights sum to 1: attn = sum(w*(v+bv)) = sum(w*v) + bv.
        bo_eff = (bo[hs].astype(np.float64)
                  + bv.astype(np.float64) @ Wo[hs].T.astype(np.float64))
        in_maps.append({
            "qT": _chunked(q[b].T),
            "kT": _chunked(k[b].T),
            "vT": _chunked(v[b].T),
            "wqT": _chunked(Wq[hs].T),
            "wkT": _chunked(Wk[hs].T),
            "wvT": _chunked(Wv[hs].T),
            "woT": _chunked(Wo[hs].T),
            "bq": np.ascontiguousarray(
                bq[hs].reshape(4, P).T).astype(np.float32),
            "bo": np.tile(bo_eff.astype(np.float32), (P, 1)),
            "tri": tri,
        })
    return in_maps


def kernel(q, k, v, mask, Wq, bq, Wk, bk, Wv, bv, Wo, bo, _trace=False):
    q, k, v = (np.asarray(x, np.float32) for x in (q, k, v))
    mask = np.asarray(mask, np.float32)
    exp_mask = np.triu(np.ones((S, S), np.float32), k=1)[None, None]
    assert mask.shape == (1, 1, S, S) and np.array_equal(mask, exp_mask), \
        "kernel specialized for the causal mask produced by setup_inputs()"

    nc = _get_nc()
    in_maps = _prep_inputs(q, k, v, Wq, bq, Wk, bk, Wv, bv, Wo, bo)
    res = run_bass_kernel_spmd(nc, in_maps, list(range(NCORES)),
                               trace=_trace)
    if _trace:
        _cache["last_result"] = res
    full = np.empty((B, S, D), np.float32)
    for c in range(NCORES):
        b, hf = divmod(c, 2)
        full[b, :, hf * HH:(hf + 1) * HH] = res.results[c]["out"]
    return full

